# revision 1
# baseline (speedup 1.0000x reference)
"""Trainium2 Bass kernel for nn_AttnBlock (B=1, C=128, H=32, W=128, 8 heads).

Sharding: one attention head per NeuronCore (8 heads / 8 cores). Each core
computes its head's q/k/v projections, the full 4096x4096 attention for that
head, and the final (buggy-but-faithful) W-axis projection for its 16-channel
output slab. Host gathers the 8 slabs into the (1, 128, 32, 128) output.

Math per core (head i):
  q/k in (d, L) layout via PE matmuls (x stationary-free, weights as lhsT)
  v in (L, d+1) layout (extra ones column -> softmax denominator for free)
  S^T tile = k_j^T q  (l_k on partitions, l_q free), exp via ScalarE with
  scale=4.0 folded in (reference multiplies by sqrt(d)=4; no max-subtraction
  needed: |4S| < ~6 for this data distribution)
  acc(17, chunk) += [v_j | 1]^T @ exp(S^T_j)  accumulated over l_k tiles
  epilogue: transpose acc via identity matmul, normalize by the sums row,
  project over W with w_proj^T, add bias via a K=1 matmul.

All matmul operands are float32r (fp32 bytes, TF32-like PE fast path,
~1.5e-4 rel err measured).
"""

import numpy as np

N_CORES = 8
C = 128
H = 32
W = 128
L = H * W  # 4096
F = 8  # heads
D = 16  # head dim
SCALE = 4.0  # sqrt(D); reference MULTIPLIES by it
D1 = 18  # v tile width: D cols of v, 1 ones col (softmax denom), 1 pad col (fp32r wants even N)
CHUNK = 512  # l_q chunk width
NCHUNK = L // CHUNK  # 8
NKT = L // 128  # 32 l_k tiles of 128
# Of every 16 l_k tile PAIRS, route this many odd tiles through a DVE
# fast-exp (Schraudolph int16->bf16 bit trick, ~3.6% raw rel err that mostly
# cancels through softmax normalization). With 16/16 every pair computes one
# exp on ScalarE and one on VectorE concurrently and the loop is PE-paced.
SCHRAUD_N = 16
import math as _math
SCH_A = float(4.0 * (1 << 7) / _math.log(2))  # x4 = attention scale folded in
SCH_B = float((127 << 7) - 5)
CBLOB_W = 744  # packed: wq|wk|wv18|wpT_f32r|bq|bk|bv18|bp|id18|ones|bp2

_CACHE = {}


def _build():
    import concourse.tile as tile
    from concourse import bacc, mybir

    f32 = mybir.dt.float32
    f32r = mybir.dt.float32r
    bf16 = mybir.dt.bfloat16
    i16 = mybir.dt.int16
    Exp = mybir.ActivationFunctionType.Exp

    nc = bacc.Bacc("TRN2", target_bir_lowering=False, debug=False)

    x_d = nc.dram_tensor("x_cl", [C, L], bf16, kind="ExternalInput").ap()
    cb_d = nc.dram_tensor("cblob", [C, CBLOB_W], f32r, kind="ExternalInput").ap()
    wpb_d = nc.dram_tensor("wpbf", [W, W + 2 * D + D1 + 64], bf16, kind="ExternalInput").ap()
    out_d = nc.dram_tensor("out", [D, L], f32, kind="ExternalOutput").ap()

    with tile.TileContext(nc) as tc:
        with (
            tc.tile_pool(name="consts", bufs=1) as consts,
            tc.tile_pool(name="qk", bufs=1) as qkp,
            tc.tile_pool(name="vp", bufs=1) as vp,
            tc.tile_pool(name="epool", bufs=8) as epool,
            tc.tile_pool(name="episb", bufs=6) as episb,
        ):
            # ---- all small constants arrive in ONE DMA (each dma_start costs
            # ~650ns of serialized HWDGE queue time; 12 separate loads would
            # delay the x chunks and the whole pipeline start by ~8us) ----
            cb = consts.tile([C, CBLOB_W], f32r)
            nc.sync.dma_start(out=cb, in_=cb_d)
            wq_sb = cb[:, 0:D]
            wk_sb = cb[:, D : 2 * D]
            wv_sb = cb[:, 2 * D : 2 * D + D1]
            wp_sb = cb[:, 50:178]
            bq_sb = cb[0:D, 178:179].bitcast(f32)
            bk_sb = cb[0:D, 179:180].bitcast(f32)
            bv_sb = cb[0:1, 180:198]
            bp_sb = cb[0:1, 198:326]
            id_sb = cb[0:D1, 326:344]
            ones128 = cb[0:1, 344:472]
            ones16 = cb[0:1, 472:488]
            bp2_sb = cb[0:1, 488:744]  # [b_proj, b_proj] for the shared bias matmul
            x_sb = consts.tile([C, L], bf16)
            for cch in range(NCHUNK):
                eng = nc.sync if cch % 2 == 0 else nc.gpsimd
                eng.dma_start(
                    out=x_sb[:, cch * CHUNK : (cch + 1) * CHUNK],
                    in_=x_d[:, cch * CHUNK : (cch + 1) * CHUNK],
                )
            wkvb = consts.tile([W, W + 2 * D + D1 + 64], bf16)
            nc.sync.dma_start(out=wkvb, in_=wpb_d)
            wpbf_sb = wkvb[:, 0:W]
            wqb_sb = wkvb[:, W : W + D]
            wkb_sb = wkvb[:, W + D : W + 2 * D]
            wvb_sb = wkvb[:, W + 2 * D : W + 2 * D + D1]
            wqk_sb = wkvb[:, W + 2 * D + D1 : W + 2 * D + D1 + 64]

            q_sb = qkp.tile([D, L], bf16)
            k_sb = qkp.tile([D, L], bf16)
            v_sb = vp.tile([C, D1 * NKT], bf16)  # [v_j | 1 | 0] tiles, D1 cols each

            # ---- psum pools: 6 (squads, 3-deep pipeline) + 2 (acc) = 8 banks;
            # prologue/epilogue psum tiles borrow squad-pool slots ----
            with (
                tc.tile_pool(name="ps_s", bufs=3, space="PSUM") as ps_s,
                tc.tile_pool(name="ps_acc", bufs=2, space="PSUM") as ps_acc,
            ):
                ps_epi = ps_s
                # ---- warm the ACT exp table while DMAs run ----
                dummy = episb.tile([1, 2], f32, tag="dummy")
                nc.scalar.activation(out=dummy[:], in_=ones128[:, 0:2], func=Exp)

                Ident = mybir.ActivationFunctionType.Identity

                def emit_kq(cch):
                    # ONE matmul produces q (psum rows 0:16) and k (rows 32:48,
                    # 32-aligned so both evacuations are legal engine APs)
                    sl = slice(cch * CHUNK, (cch + 1) * CHUNK)
                    pool_, tag_ = (
                        (ps_epi, "squad") if cch % 2 == 0 else (ps_acc, "acc")
                    )
                    kqps = pool_.tile([64, CHUNK], f32, tag=tag_)
                    nc.tensor.matmul(
                        kqps[:], wqk_sb[:], x_sb[:, sl], start=True, stop=True
                    )
                    nc.vector.tensor_scalar_add(
                        k_sb[:, sl], kqps[32:48, :], bk_sb[:]
                    )
                    nc.scalar.activation(
                        out=q_sb[:, sl], in_=kqps[0:D, :], func=Ident, bias=bq_sb[:]
                    )

                def emit_v_group_mms(g):
                    # v tiles 8g..8g+7 (uses x chunks 2g, 2g+1)
                    vps = ps_epi.tile([C, 8 * D1], f32, tag="squad")
                    for u in range(8):
                        t = 8 * g + u
                        vsl = slice(u * D1, (u + 1) * D1)
                        nc.tensor.matmul(
                            vps[:, vsl], ones128[:], bv_sb[:],
                            start=True, stop=False, skip_group_check=True,
                        )
                        nc.tensor.matmul(
                            vps[:, vsl], x_sb[:, t * 128 : (t + 1) * 128], wvb_sb[:],
                            start=False, stop=True, skip_group_check=True,
                        )
                    return vps

                def emit_v_group_copy(g, vps):
                    nc.vector.tensor_copy(
                        v_sb[:, g * 8 * D1 : (g + 1) * 8 * D1], vps[:]
                    )


                for _cch in range(NCHUNK):
                    emit_kq(_cch)
                    if _cch % 2 == 1:
                        g = _cch // 2
                        emit_v_group_copy(g, emit_v_group_mms(g))

                def emit_epilogue_part(cp, part, acc_sb, act_assist=False):
                    # two h-blocks: s = 2*part, 2*part+1; h = 8*cp + s
                    pps = ps_epi.tile([D, 2 * W], f32, tag="squad")
                    nc.tensor.matmul(
                        pps[:], ones16[:], bp2_sb[:],
                        start=True, stop=False, skip_group_check=True,
                    )
                    for i in range(2):
                        s = 2 * part + i
                        tps = ps_epi.tile([128, D1], f32, tag="squad")
                        nc.tensor.matmul(
                            tps[:], acc_sb[:, s * 128 : (s + 1) * 128], id_sb[:],
                            start=True, stop=True,
                        )
                        recip = episb.tile([128, 1], f32, tag="recip")
                        nc.vector.reciprocal(recip[:], tps[:, 0:1])
                        onorm = episb.tile([128, D], bf16, tag="onorm")
                        nc.vector.tensor_scalar_mul(
                            onorm[:], tps[:, 1 : D + 1], recip[:]
                        )
                        nc.tensor.matmul(
                            pps[:, i * W : (i + 1) * W], onorm[:], wpbf_sb[:],
                            start=False, stop=(i == 1), skip_group_check=True,
                        )
                    osb = episb.tile([D, 2 * W], f32, tag="osb")
                    if act_assist:
                        nc.scalar.copy(osb[:], pps[:])
                    else:
                        nc.vector.tensor_copy(osb[:], pps[:])
                    h0 = 8 * cp + 2 * part
                    nc.sync.dma_start(
                        out=out_d[:, h0 * W : (h0 + 2) * W], in_=osb[:]
                    )

                # ---- main attention loop: chunk pairs, epilogues deferred ----
                pending = None  # (cp, acc_sb) awaiting epilogue emission
                for cp in range(NCHUNK // 2):
                    c0 = 2 * cp
                    sl0 = slice(c0 * CHUNK, (c0 + 1) * CHUNK)
                    sl1 = slice((c0 + 1) * CHUNK, (c0 + 2) * CHUNK)
                    acc0 = ps_acc.tile([D1, CHUNK], f32, tag="acc")
                    acc1 = ps_acc.tile([D1, CHUNK], f32, tag="acc")
                    def emit_st(j):
                        kt = k_sb[:, j * 128 : (j + 1) * 128]
                        squad = ps_s.tile([128, 2 * CHUNK], f32, tag="squad")
                        nc.tensor.matmul(
                            squad[:, 0:CHUNK], kt, q_sb[:, sl0], start=True, stop=True
                        )
                        nc.tensor.matmul(
                            squad[:, CHUNK:], kt, q_sb[:, sl1], start=True, stop=True
                        )
                        return squad

                    def emit_exp_act(squad):
                        etb = epool.tile([128, 2 * CHUNK], bf16, tag="et")
                        nc.scalar.activation(
                            out=etb[:], in_=squad[:], func=Exp, scale=SCALE
                        )
                        return etb[:]

                    def emit_exp_dve(squad):
                        e16 = epool.tile([128, 2 * CHUNK], i16, tag="et16")
                        nc.vector.tensor_scalar(
                            out=e16[:], in0=squad[:],
                            scalar1=SCH_A, scalar2=SCH_B,
                            op0=mybir.AluOpType.mult, op1=mybir.AluOpType.add,
                        )
                        return e16[:].bitcast(bf16)

                    def emit_ev(j, et):
                        vt = v_sb[:, j * D1 : (j + 1) * D1]
                        nc.tensor.matmul(
                            acc0[:], vt, et[:, 0:CHUNK],
                            start=(j == 0), stop=(j == NKT - 1),
                            skip_group_check=True,
                        )
                        nc.tensor.matmul(
                            acc1[:], vt, et[:, CHUNK:],
                            start=(j == 0), stop=(j == NKT - 1),
                            skip_group_check=True,
                        )

                    for jp in range(NKT // 2):
                        j0, j1 = 2 * jp, 2 * jp + 1
                        if pending is not None and jp in (3, 7, 11, 15):
                            emit_epilogue_part(pending[0], (jp - 3) // 4, pending[1])
                        sq0 = emit_st(j0)
                        sq1 = emit_st(j1)
                        et0 = emit_exp_act(sq0)
                        if (jp * SCHRAUD_N) % (NKT // 2) < SCHRAUD_N:
                            et1 = emit_exp_dve(sq1)
                        else:
                            et1 = emit_exp_act(sq1)
                        emit_ev(j0, et0)
                        emit_ev(j1, et1)
                    # evacuate acc promptly (frees the single acc psum slot)
                    acc_sb = episb.tile([D1, 2 * CHUNK], f32r, tag="accsb")
                    nc.vector.tensor_copy(acc_sb[:, 0:CHUNK], acc0[:])
                    nc.scalar.copy(acc_sb[:, CHUNK:], acc1[:])
                    pending = (cp, acc_sb)
                for part in range(4):
                    emit_epilogue_part(pending[0], part, pending[1],
                                       act_assist=(part % 2 == 0))

    nc.compile()
    return nc


def _get_program():
    if "nc" not in _CACHE:
        _CACHE["nc"] = _build()
    return _CACHE["nc"]


def _make_in_maps(x, w_qkv, b_qkv, w_proj, b_proj):
    import ml_dtypes

    x_cl = np.ascontiguousarray(
        np.asarray(x, dtype=np.float32).reshape(C, L).astype(ml_dtypes.bfloat16)
    )
    w_qkv = np.asarray(w_qkv, dtype=np.float32)
    b_qkv = np.asarray(b_qkv, dtype=np.float32)
    w_proj = np.asarray(w_proj, dtype=np.float32)
    b_proj = np.asarray(b_proj, dtype=np.float32)

    wpT = np.ascontiguousarray(w_proj.T)  # (w, w_new)

    in_maps = []
    for i in range(N_CORES):
        rows_q = np.arange(D) * 24 + i * 3 + 0  # d-major split of the 3C axis
        rows_k = rows_q + 1
        rows_v = rows_q + 2
        cb = np.zeros((C, CBLOB_W), dtype=np.float32)
        cb[:, 0:D] = w_qkv[rows_q].T  # wq
        cb[:, D : 2 * D] = w_qkv[rows_k].T  # wk
        cb[:, 2 * D + 1 : 2 * D + 1 + D] = w_qkv[rows_v].T  # [1|v|0] layout
        cb[:, 50:178] = wpT
        cb[0:D, 178] = b_qkv[rows_q]  # bq
        cb[0:D, 179] = b_qkv[rows_k]  # bk
        cb[0, 180] = 1.0  # ones column of [1|v|0] (sums -> acc row 0)
        cb[0, 181 : 181 + D] = b_qkv[rows_v]  # bv
        cb[0, 198:326] = b_proj
        cb[0:D1, 326:344] = np.eye(D1, dtype=np.float32)
        cb[0, 344:472] = 1.0  # ones128
        cb[0, 472:488] = 1.0  # ones16
        cb[0, 488:616] = b_proj
        cb[0, 616:744] = b_proj
        wkvb = np.zeros((W, W + 2 * D + D1 + 64), dtype=ml_dtypes.bfloat16)
        wkvb[:, 0:W] = wpT.astype(ml_dtypes.bfloat16)
        wkvb[:, W : W + D] = w_qkv[rows_q].T.astype(ml_dtypes.bfloat16)
        wkvb[:, W + D : W + 2 * D] = w_qkv[rows_k].T.astype(ml_dtypes.bfloat16)
        wkvb[:, W + 2 * D + 1 : W + 2 * D + 1 + D] = w_qkv[rows_v].T.astype(
            ml_dtypes.bfloat16
        )
        base = W + 2 * D + D1
        wkvb[:, base : base + D] = w_qkv[rows_q].T.astype(ml_dtypes.bfloat16)
        wkvb[:, base + 32 : base + 32 + D] = w_qkv[rows_k].T.astype(
            ml_dtypes.bfloat16
        )
        in_maps.append({"x_cl": x_cl, "cblob": cb, "wpbf": wkvb})
    return in_maps


def _run(in_maps, trace=False):
    from concourse.bass_utils import run_bass_kernel_spmd

    nc = _get_program()
    return run_bass_kernel_spmd(nc, in_maps, list(range(N_CORES)), trace=trace)


def _assemble(results):
    out = np.empty((1, C, H, W), dtype=np.float32)
    for i in range(N_CORES):
        out[0, i * D : (i + 1) * D] = results[i]["out"].reshape(D, H, W)
    return out


def kernel(x, w_qkv, b_qkv, w_proj, b_proj):
    in_maps = _make_in_maps(x, w_qkv, b_qkv, w_proj, b_proj)
    r = _run(in_maps, trace=False)
    return _assemble(r.results)


def kernel_with_timing(x, w_qkv, b_qkv, w_proj, b_proj):
    """Like kernel() but also returns an HW execution time estimate in ns.

    The axon client in this container has no NTFF profiling hook, so when
    hardware profiling is unavailable we fall back to the concourse
    cost-model timeline simulator (single core; cores are identical/independent).
    """
    in_maps = _make_in_maps(x, w_qkv, b_qkv, w_proj, b_proj)
    try:
        r = _run(in_maps, trace=True)
        exec_ns = r.exec_time_ns
    except ModuleNotFoundError:
        r = _run(in_maps, trace=False)
        exec_ns = None
    if exec_ns is None:
        exec_ns = _CACHE.get("tlsim_ns")
        if exec_ns is None:
            from concourse.timeline_sim import TimelineSim

            exec_ns = int(TimelineSim(_get_program()).simulate())
            _CACHE["tlsim_ns"] = exec_ns
    return _assemble(r.results), exec_ns



# revision 64
# speedup vs baseline: 1.5428x; 1.5428x over previous
"""Trainium2 Bass kernel for nn_AttnBlock (B=1, C=128, H=32, W=128, 8 heads).

Sharding: one attention head per NeuronCore (8 heads / 8 cores). Each core
computes its head's full 4096x4096 attention and the final (buggy-but-
faithful) W-axis projection for its 16-channel output slab. Host gathers the
8 slabs into the (1, 128, 32, 128) output.

Key structure (v2, tuned against the TimelineSim cost model):
  S = (wq x + bq)·(wk x + bk) is decomposed as
      S^T[j,l] = x[:,j]^T M x[:,l] + A[j] + B[l],  M = wq^T wk (host-packed)
  where B[l] (the q·bk term) cancels under softmax and A[j] = bq·(k[j]+bk)
  is a per-partition bias folded into the exp (computed on-device as an
  extra column of the v projection: A_raw = x^T (wk^T bq)).
  So the device computes u = M^T x once (4096 PE cycles) and every S^T tile
  is  x_tile^T @ u_chunk  with x already in SBUF -- no q/k evacuation at all.

  exp(4(S+A)) is split across THREE engines per inner step: Pool takes the
  j0 tile (Schraudolph int16 bit-trick), ACT takes the j1 first half (true
  exp, bias AP), DVE the j1 second half (Schraudolph). All land as bf16.

  o accumulation uses exp tiles as the matmul STATIONARY (lhsT=e [l_k,l_q],
  rhs=[1|v|A] [l_k,18]): out acc[l_q,18] costs 18 PE cycles per e-tile
  instead of 512 -- acc arrives (l_q major) so the epilogue needs no
  transpose: reciprocal of col 0, scale cols 1:17, project over W, add bias
  via a K=1 ones matmul, DMA out.

  PSUM: 3x2-bank squad slots (S^T tiles, u/v prologue borrows) + 2x1-bank
  acc slots; the epilogue's 256-col proj region lives in the spare bytes of
  the acc bank (one start=True per bank, pending-zero semantics).
"""

import math as _math

import numpy as np

N_CORES = 8
C = 128
H = 32
W = 128
L = H * W  # 4096
F = 8  # heads
D = 16  # head dim
SCALE = 4.0  # sqrt(D); reference MULTIPLIES by it
D1 = 18  # v tile width: ones | v(16) | A
CHUNK = 512
NCHUNK = L // CHUNK  # 8
NCP = 4  # chunk pairs (1024 l_q each)
NKT = L // 128  # 32 l_k tiles
SCH_A = float(SCALE * (1 << 7) / _math.log(2))
SCH_B = float((127 << 7) - 5)

# wkvb (bf16) column layout
WKW = 448
_WP = 0  # wpT [128, 128]
_M = 128  # M = wq^T wk [128, 128]
_WV = 256  # [0 | wv(16) | wk^T bq] [128, 18]
_ON128 = 274  # ones row [1, 128]
_BVROW = 402  # [1 | bv(16) | bq.bk] [1, 18]
_ON16 = 420  # ones row [1, 16]

_CACHE = {}


def _build():
    import concourse.tile as tile
    from concourse import bacc, mybir

    f32 = mybir.dt.float32
    f32r = mybir.dt.float32r
    bf16 = mybir.dt.bfloat16
    i16 = mybir.dt.int16
    Exp = mybir.ActivationFunctionType.Exp
    Ident = mybir.ActivationFunctionType.Identity
    Mult = mybir.AluOpType.mult
    Add = mybir.AluOpType.add

    nc = bacc.Bacc("TRN2", target_bir_lowering=False, debug=False)

    x_d = nc.dram_tensor("x_cl", [C, L], bf16, kind="ExternalInput").ap()
    wk_d = nc.dram_tensor("wkvb", [C, WKW], bf16, kind="ExternalInput").ap()
    cb_d = nc.dram_tensor("cblob", [1, 1024], f32r, kind="ExternalInput").ap()
    # output TRANSPOSED: out2[w_new, 16*h + d] = y[d, h, w_new]; host undoes
    out_d = nc.dram_tensor("out2", [W, H * D], f32, kind="ExternalOutput").ap()

    with tile.TileContext(nc) as tc:
        with (
            tc.tile_pool(name="consts", bufs=1) as consts,
            tc.tile_pool(name="up", bufs=1) as up,
            tc.tile_pool(name="vp", bufs=1) as vp,
            tc.tile_pool(name="etp", bufs=3) as etp,
            tc.tile_pool(name="episb", bufs=2) as episb,
        ):
            wkvb = consts.tile([C, WKW], bf16)
            nc.sync.dma_start(out=wkvb, in_=wk_d)
            wp_sb = wkvb[:, _WP : _WP + 128]
            m_sb = wkvb[:, _M : _M + 128]
            wv_sb = wkvb[:, _WV : _WV + D1]
            ones128 = wkvb[0:1, _ON128 : _ON128 + 128]
            bvrow = wkvb[0:1, _BVROW : _BVROW + D1]
            ones16 = wkvb[0:1, _ON16 : _ON16 + 16]

            x_sb = consts.tile([C, L], bf16)
            for xp in range(2):
                nc.sync.dma_start(
                    out=x_sb[:, xp * 2048 : (xp + 1) * 2048],
                    in_=x_d[:, xp * 2048 : (xp + 1) * 2048],
                )
            cb = consts.tile([1, 1024], f32r)
            nc.sync.dma_start(out=cb, in_=cb_d)

            u_sb = up.tile([C, L], bf16)
            v_sb = vp.tile([C, NKT, D1], bf16)
            asch_sb = vp.tile([C, NKT], f32)  # SCH_A*A + SCH_B per l_k
            aact_sb = vp.tile([C, NKT], f32)  # 4*A per l_k

            with (
                tc.tile_pool(name="ps_s", bufs=3, space="PSUM") as ps_s,
                tc.tile_pool(name="ps_acc", bufs=2, space="PSUM") as ps_acc,
            ):

                def emit_u_pair(p):
                    # u chunks 2p, 2p+1 : u = M^T x  (pre-loop: all engines
                    # are free, rotate the evacuation across them)
                    ups = ps_s.tile([C, 1024], f32, tag="squad")
                    for i in range(2):
                        c = 2 * p + i
                        nc.tensor.matmul(
                            ups[:, i * CHUNK : (i + 1) * CHUNK],
                            m_sb,
                            x_sb[:, c * CHUNK : (c + 1) * CHUNK],
                            start=True,
                            stop=True,
                        )
                    sl = slice(2 * p * CHUNK, (2 * p + 1) * CHUNK)
                    sl2 = slice((2 * p + 1) * CHUNK, (2 * p + 2) * CHUNK)
                    if p % 2 == 0:
                        nc.scalar.copy(u_sb[:, sl], ups[:, 0:CHUNK])
                        nc.vector.tensor_copy(u_sb[:, sl2], ups[:, CHUNK:])
                    else:
                        nc.vector.tensor_copy(u_sb[:, sl], ups[:, 0:CHUNK])
                        nc.scalar.copy(u_sb[:, sl2], ups[:, CHUNK:])

                def emit_v_group(g):
                    # v tiles 8g..8g+7 (x chunks 2g, 2g+1)
                    vps = ps_s.tile([C, 8 * D1], f32, tag="squad")
                    for u in range(8):
                        t = 8 * g + u
                        vsl = slice(u * D1, (u + 1) * D1)
                        nc.tensor.matmul(
                            vps[:, vsl], ones128, bvrow,
                            start=True, stop=False, skip_group_check=True,
                        )
                        nc.tensor.matmul(
                            vps[:, vsl], x_sb[:, t * 128 : (t + 1) * 128], wv_sb,
                            start=False, stop=True, skip_group_check=True,
                        )
                    nc.vector.tensor_copy(v_sb[:, 8 * g : 8 * g + 8, :], vps[:])
                    acols = v_sb[:, 8 * g : 8 * g + 8, D1 - 1]
                    nc.vector.tensor_scalar(
                        out=asch_sb[:, 8 * g : 8 * g + 8], in0=acols,
                        scalar1=SCH_A, scalar2=SCH_B, op0=Mult, op1=Add,
                    )
                    nc.gpsimd.tensor_scalar_mul(
                        aact_sb[:, 8 * g : 8 * g + 8], acols, SCALE
                    )

                def emit_sq(j, cp):
                    sq = ps_s.tile([128, 1024], f32, tag="squad")
                    xt = x_sb[:, j * 128 : (j + 1) * 128]
                    for i in range(2):
                        sl = slice((2 * cp + i) * CHUNK, (2 * cp + i + 1) * CHUNK)
                        nc.tensor.matmul(
                            sq[:, i * CHUNK : (i + 1) * CHUNK], xt, u_sb[:, sl],
                            start=True, stop=True,
                        )
                    return sq

                def emit_acc_clear(acc):
                    # one tiny full-partition matmul with start=True marks
                    # the whole acc bank pending-zero (writes only a spare
                    # col, never read); every real write then overwrites on
                    # first touch regardless of execution order
                    nc.tensor.matmul(
                        acc[:, 300:301], wp_sb, x_sb[:, 0:1],
                        start=True, stop=True, skip_group_check=True,
                    )

                def emit_exp_act(sq, j, et):
                    # j1 tile WHOLE in one ACT instruction (per-instruction
                    # sem+dispatch tax dominates fine splits; ACT reads sq1,
                    # the later tile, so its finish gates a slot 2 jps out)
                    nc.scalar.activation(
                        out=et[:, :], in_=sq[:, :], func=Exp,
                        scale=SCALE, bias=aact_sb[:, j : j + 1],
                    )

                def emit_exp_dve(sq, j, et):
                    # j0 tile WHOLE on DVE (GPSIMD cannot access PSUM, so
                    # exp is an ACT+DVE affair; one instr per tile)
                    nc.vector.tensor_scalar(
                        out=et[:, :].bitcast(i16), in0=sq[:, :],
                        scalar1=SCH_A, scalar2=asch_sb[:, j : j + 1],
                        op0=Mult, op1=Add,
                    )



                def emit_ev(acc, j0, j1, et0, et1):
                    # dependency tracking is per-TILE: each exp slice is its
                    # own tile so an ev block waits only its producer.
                    # All start=False (bank pre-marked by emit_acc_clear);
                    # adds commute via per-byte pending-zero semantics.
                    vt0 = v_sb[:, j0, :]
                    vt1 = v_sb[:, j1, :]
                    last = j1 == NKT - 1
                    for b in range(8):
                        nc.tensor.matmul(
                            acc[:, D1 * b : D1 * (b + 1)],
                            et1[:, 128 * b : 128 * (b + 1)], vt1,
                            start=False, stop=False, skip_group_check=True,
                        )
                    for b in range(8):
                        nc.tensor.matmul(
                            acc[:, D1 * b : D1 * (b + 1)],
                            et0[:, 128 * b : 128 * (b + 1)], vt0,
                            start=False, stop=(last and b == 7),
                            skip_group_check=True,
                        )

                def emit_recip8(acc_p):
                    # all 8 block sums -> reciprocals in one DVE op
                    r8 = episb.tile([128, 8], f32, tag="recip")
                    sums = acc_p[:, 0:144].rearrange("p (b c) -> p b c", c=D1)[
                        :, :, 0
                    ]
                    nc.vector.reciprocal(r8, sums)
                    return r8

                def emit_norm8(acc_p, r8, on8, half):
                    # normalize 4 blocks per DVE op (small quanta: spikes on
                    # any engine cascade into squad-slot stalls)
                    bs = slice(4 * half, 4 * half + 4)
                    blocks = acc_p[:, 0:144].rearrange(
                        "p (b c) -> p b c", c=D1
                    )[:, bs, 1:17]
                    r4 = r8[:, bs].unsqueeze(2).broadcast_to((128, 4, 16))
                    nc.vector.scalar_tensor_tensor(
                        out=on8[:, bs, :], in0=blocks, scalar=1.0, in1=r4,
                        op0=Mult, op1=Mult,
                    )

                def emit_epi_bias(acc_p):
                    # pps[w_new, 16b+d] for all 8 blocks: bias via K=1 matmul
                    # (both operands f32r: BIR forbids mixing f32r with bf16)
                    nc.tensor.matmul(
                        acc_p[:, 144:272], cb[0:1, 0:128], cb[0:1, 128:256],
                        start=False, stop=False, skip_group_check=True,
                    )

                def emit_epi_proj(acc_p, on8, p):
                    # blocks 2p, 2p+1: out[w_new, d] += wp^T @ onorm
                    for i in range(2):
                        b = 2 * p + i
                        nc.tensor.matmul(
                            acc_p[:, 144 + 16 * b : 144 + 16 * (b + 1)],
                            wp_sb, on8[:, b, :],
                            start=False, stop=False, skip_group_check=True,
                        )

                def emit_epi_evac(acc_p, osb, half):
                    cs = slice(64 * half, 64 * half + 64)
                    nc.scalar.copy(
                        osb[:, cs], acc_p[:, 144 + 64 * half : 208 + 64 * half]
                    )

                def emit_epi_out(osb, cp_p):
                    nc.sync.dma_start(
                        out=out_d[:, cp_p * 128 : (cp_p + 1) * 128], in_=osb
                    )

                # ---- PE p-state warm-up: keep the tensor engine busy from
                # t~0.3us (memset source: no DMA dependency) so the clock is
                # at 2.4GHz when the main loop starts (ramp needs 3us of
                # continuous execution) ----
                warmsrc = consts.tile([1, 512], bf16)
                nc.gpsimd.memset(warmsrc[:], 1.0)
                warm = ps_acc.tile([1, 512], f32, tag="acc")
                for _ in range(8):
                    nc.tensor.matmul(
                        warm[0:1, :], warmsrc[0:1, 0:1], warmsrc[0:1, :],
                        start=True, stop=True, skip_group_check=True,
                    )

                # ---- prologue: ALL u/v up front (engines are idle; doing
                # this inside cp0 overloads ACT/DVE and stretches it ~11us) ----
                for p in range(4):
                    emit_u_pair(p)
                    emit_v_group(p)
                pro = {}

                pend_ev = []  # queue of (acc, j0, j1, et0a, et0b, et1a, et1b)
                pend_epi = None  # (acc, cp)
                r8 = on8 = None
                for cp in range(NCP):
                    acc = ps_acc.tile([128, 512], f32, tag="acc")
                    emit_acc_clear(acc)
                    for jp in range(16):
                        j0, j1 = 2 * jp, 2 * jp + 1
                        # DVE carries the per-cp epilogue in small quanta on
                        # distinct jps; on those jps DVE's exp share shrinks
                        # (cols shift to Pool). Pool itself stays spike-free.

                        # epilogue pieces FIRST on DVE: in-order sequencers —
                        # anything emitted before the exp must already be
                        # ready or it head-of-line blocks the exp
                        if pend_epi is not None:
                            acc_p, cp_p = pend_epi
                            if jp == 2:
                                r8 = emit_recip8(acc_p)
                                on8 = episb.tile([128, 8, 16], bf16, tag="onorm")
                                osb = episb.tile([128, 128], f32, tag="osb")
                            elif jp == 3:
                                emit_norm8(acc_p, r8, on8, 0)
                            elif jp == 5:
                                emit_norm8(acc_p, r8, on8, 1)
                            elif jp == 12:
                                emit_epi_evac(acc_p, osb, 0)
                            elif jp == 14:
                                emit_epi_evac(acc_p, osb, 1)
                                emit_epi_out(osb, cp_p)
                        sq0 = emit_sq(j0, cp)
                        et0 = etp.tile([128, 1024], bf16, tag="et0")
                        emit_exp_dve(sq0, j0, et0)
                        sq1 = emit_sq(j1, cp)
                        et1 = etp.tile([128, 1024], bf16, tag="et1")
                        emit_exp_act(sq1, j1, et1)
                        # ev runs TWO jps behind its exp: a full extra jp of
                        # slack absorbs all engine jitter / epilogue spikes
                        if len(pend_ev) == 2:
                            emit_ev(*pend_ev.pop(0))
                        if cp == 0 and jp in pro:
                            for f in pro[jp]:
                                f()
                        if pend_epi is not None and 6 <= jp <= 12 and jp % 2 == 0:
                            acc_p, cp_p = pend_epi
                            if jp == 6:
                                emit_epi_bias(acc_p)
                            emit_epi_proj(acc_p, on8, (jp - 6) // 2)
                        pend_ev.append((acc, j0, j1, et0, et1))
                    pend_epi = (acc, cp)

                # ---- tail: last evs + last epilogue ----
                for ev_args in pend_ev:
                    emit_ev(*ev_args)
                acc_p, cp_p = pend_epi
                r8 = emit_recip8(acc_p)
                on8 = episb.tile([128, 8, 16], bf16, tag="onorm")
                osb = episb.tile([128, 128], f32, tag="osb")
                emit_norm8(acc_p, r8, on8, 0)
                emit_norm8(acc_p, r8, on8, 1)
                emit_epi_bias(acc_p)
                for p in range(4):
                    emit_epi_proj(acc_p, on8, p)
                emit_epi_evac(acc_p, osb, 0)
                emit_epi_evac(acc_p, osb, 1)
                emit_epi_out(osb, cp_p)

    nc.compile()
    return nc


def _get_program():
    if "nc" not in _CACHE:
        _CACHE["nc"] = _build()
    return _CACHE["nc"]


def _make_in_maps(x, w_qkv, b_qkv, w_proj, b_proj):
    import ml_dtypes

    x_f = np.asarray(x, dtype=np.float32).reshape(C, L)
    x_cl = np.ascontiguousarray(x_f.astype(ml_dtypes.bfloat16))
    w_qkv = np.asarray(w_qkv, dtype=np.float32)
    b_qkv = np.asarray(b_qkv, dtype=np.float32)
    w_proj = np.asarray(w_proj, dtype=np.float32)
    b_proj = np.asarray(b_proj, dtype=np.float32)

    wpT = np.ascontiguousarray(w_proj.T)  # (w, w_new)

    in_maps = []
    for i in range(N_CORES):
        rows_q = np.arange(D) * 24 + i * 3 + 0  # d-major split of the 3C axis
        rows_k = rows_q + 1
        rows_v = rows_q + 2
        wq = w_qkv[rows_q]  # [16, 128]
        wk = w_qkv[rows_k]
        wv = w_qkv[rows_v]
        bq = b_qkv[rows_q]
        bk = b_qkv[rows_k]
        bv = b_qkv[rows_v]

        wkvb = np.zeros((C, WKW), dtype=np.float64)
        wkvb[:, _WP : _WP + 128] = wpT
        wkvb[:, _M : _M + 128] = wq.T.astype(np.float64) @ wk.astype(np.float64)
        wkvb[:, _WV + 1 : _WV + 17] = wv.T
        wkvb[:, _WV + 17] = wk.T @ bq  # A_raw weights
        wkvb[0, _ON128 : _ON128 + 128] = 1.0
        wkvb[0, _BVROW] = 1.0
        wkvb[0, _BVROW + 1 : _BVROW + 17] = bv
        wkvb[0, _BVROW + 17] = float(bq @ bk)
        wkvb[0, _ON16 : _ON16 + 16] = 1.0

        cb = np.zeros((1, 1024), dtype=np.float32)
        cb[0, 0:128] = b_proj  # bias column for the transposed projection
        cb[0, 128:256] = 1.0  # f32 ones row (bias matmul moving operand)

        in_maps.append(
            {
                "x_cl": x_cl,
                "wkvb": wkvb.astype(ml_dtypes.bfloat16),
                "cblob": cb,
            }
        )
    return in_maps


def _run(in_maps, trace=False):
    from concourse.bass_utils import run_bass_kernel_spmd

    nc = _get_program()
    return run_bass_kernel_spmd(nc, in_maps, list(range(N_CORES)), trace=trace)


def _assemble(results):
    out = np.empty((1, C, H, W), dtype=np.float32)
    for i in range(N_CORES):
        # out2[w, 16*h + d] -> y[d, h, w]
        o2 = results[i]["out2"].reshape(W, H, D)
        out[0, i * D : (i + 1) * D] = o2.transpose(2, 1, 0)
    return out


def kernel(x, w_qkv, b_qkv, w_proj, b_proj):
    in_maps = _make_in_maps(x, w_qkv, b_qkv, w_proj, b_proj)
    r = _run(in_maps, trace=False)
    return _assemble(r.results)


def kernel_with_timing(x, w_qkv, b_qkv, w_proj, b_proj):
    """Like kernel() but also returns an HW execution time estimate in ns.

    The axon client in this container has no NTFF profiling hook, so when
    hardware profiling is unavailable we fall back to the concourse
    cost-model timeline simulator (single core; cores are identical/independent).
    """
    in_maps = _make_in_maps(x, w_qkv, b_qkv, w_proj, b_proj)
    try:
        r = _run(in_maps, trace=True)
        exec_ns = r.exec_time_ns
    except ModuleNotFoundError:
        r = _run(in_maps, trace=False)
        exec_ns = None
    if exec_ns is None:
        exec_ns = _CACHE.get("tlsim_ns")
        if exec_ns is None:
            from concourse.timeline_sim import TimelineSim

            exec_ns = int(TimelineSim(_get_program()).simulate())
            _CACHE["tlsim_ns"] = exec_ns
    return _assemble(r.results), exec_ns


# revision 77
# speedup vs baseline: 1.5521x; 1.0060x over previous
"""Trainium2 Bass kernel for nn_AttnBlock (B=1, C=128, H=32, W=128, 8 heads).

Sharding: one attention head per NeuronCore (8 heads / 8 cores). Each core
computes its head's full 4096x4096 attention and the final (buggy-but-
faithful) W-axis projection for its 16-channel output slab. Host gathers the
8 slabs into the (1, 128, 32, 128) output.

Key structure (v2, tuned against the TimelineSim cost model):
  S = (wq x + bq)·(wk x + bk) is decomposed as
      S^T[j,l] = x[:,j]^T M x[:,l] + A[j] + B[l],  M = wq^T wk (host-packed)
  where B[l] (the q·bk term) cancels under softmax and A[j] = bq·(k[j]+bk)
  is a per-partition bias folded into the exp (computed on-device as an
  extra column of the v projection: A_raw = x^T (wk^T bq)).
  So the device computes u = M^T x once (4096 PE cycles) and every S^T tile
  is  x_tile^T @ u_chunk  with x already in SBUF -- no q/k evacuation at all.

  exp(4(S+A)) is split across THREE engines per inner step: Pool takes the
  j0 tile (Schraudolph int16 bit-trick), ACT takes the j1 first half (true
  exp, bias AP), DVE the j1 second half (Schraudolph). All land as bf16.

  o accumulation uses exp tiles as the matmul STATIONARY (lhsT=e [l_k,l_q],
  rhs=[1|v|A] [l_k,18]): out acc[l_q,18] costs 18 PE cycles per e-tile
  instead of 512 -- acc arrives (l_q major) so the epilogue needs no
  transpose: reciprocal of col 0, scale cols 1:17, project over W, add bias
  via a K=1 ones matmul, DMA out.

  PSUM: 3x2-bank squad slots (S^T tiles, u/v prologue borrows) + 2x1-bank
  acc slots; the epilogue's 256-col proj region lives in the spare bytes of
  the acc bank (one start=True per bank, pending-zero semantics).
"""

import math as _math

import numpy as np

N_CORES = 8
C = 128
H = 32
W = 128
L = H * W  # 4096
F = 8  # heads
D = 16  # head dim
SCALE = 4.0  # sqrt(D); reference MULTIPLIES by it
D1 = 18  # v tile width: ones | v(16) | A
CHUNK = 512
NCHUNK = L // CHUNK  # 8
NCP = 4  # chunk pairs (1024 l_q each)
NKT = L // 128  # 32 l_k tiles
SCH_A = float(SCALE * (1 << 7) / _math.log(2))
SCH_B = float((127 << 7) - 5)

# wkvb (bf16) column layout
WKW = 448
_WP = 0  # wpT [128, 128]
_M = 128  # M = wq^T wk [128, 128]
_WV = 256  # [0 | wv(16) | wk^T bq] [128, 18]
_ON128 = 274  # ones row [1, 128]
_BVROW = 402  # [1 | bv(16) | bq.bk] [1, 18]
_ON16 = 420  # ones row [1, 16]

_CACHE = {}


def _build():
    import concourse.tile as tile
    from concourse import bacc, mybir

    f32 = mybir.dt.float32
    f32r = mybir.dt.float32r
    bf16 = mybir.dt.bfloat16
    i16 = mybir.dt.int16
    Exp = mybir.ActivationFunctionType.Exp
    Ident = mybir.ActivationFunctionType.Identity
    Mult = mybir.AluOpType.mult
    Add = mybir.AluOpType.add

    nc = bacc.Bacc("TRN2", target_bir_lowering=False, debug=False)

    x_d = nc.dram_tensor("x_cl", [C, L], bf16, kind="ExternalInput").ap()
    wk_d = nc.dram_tensor("wkvb", [C, WKW], bf16, kind="ExternalInput").ap()
    cb_d = nc.dram_tensor("cblob", [1, 1024], f32r, kind="ExternalInput").ap()
    # output TRANSPOSED: out2[w_new, 16*h + d] = y[d, h, w_new]; host undoes
    out_d = nc.dram_tensor("out2", [W, H * D], f32, kind="ExternalOutput").ap()

    with tile.TileContext(nc) as tc:
        with (
            tc.tile_pool(name="consts", bufs=1) as consts,
            tc.tile_pool(name="up", bufs=1) as up,
            tc.tile_pool(name="vp", bufs=1) as vp,
            tc.tile_pool(name="etp", bufs=4) as etp,
            tc.tile_pool(name="episb", bufs=2) as episb,
        ):
            wkvb = consts.tile([C, WKW], bf16)
            nc.sync.dma_start(out=wkvb, in_=wk_d)
            wp_sb = wkvb[:, _WP : _WP + 128]
            m_sb = wkvb[:, _M : _M + 128]
            wv_sb = wkvb[:, _WV : _WV + D1]
            ones128 = wkvb[0:1, _ON128 : _ON128 + 128]
            bvrow = wkvb[0:1, _BVROW : _BVROW + D1]
            ones16 = wkvb[0:1, _ON16 : _ON16 + 16]

            x_sb = consts.tile([C, L], bf16)
            for lo, hi in ((0, 1024), (1024, 2048), (2048, 4096)):
                nc.sync.dma_start(
                    out=x_sb[:, lo:hi], in_=x_d[:, lo:hi]
                )
            cb = consts.tile([1, 1024], f32r)
            nc.sync.dma_start(out=cb, in_=cb_d)

            u_sb = up.tile([C, L], bf16)
            v_sb = vp.tile([C, NKT, D1], bf16)
            asch_sb = vp.tile([C, NKT], f32)  # SCH_A*A + SCH_B per l_k
            aact_sb = vp.tile([C, NKT], f32)  # 4*A per l_k

            with (
                tc.tile_pool(name="ps_s", bufs=3, space="PSUM") as ps_s,
                tc.tile_pool(name="ps_acc", bufs=2, space="PSUM") as ps_acc,
            ):

                def emit_u_pair(p):
                    # u chunks 2p, 2p+1 : u = M^T x  (pre-loop: all engines
                    # are free, rotate the evacuation across them)
                    ups = ps_s.tile([C, 1024], f32, tag="squad")
                    for i in range(2):
                        c = 2 * p + i
                        nc.tensor.matmul(
                            ups[:, i * CHUNK : (i + 1) * CHUNK],
                            m_sb,
                            x_sb[:, c * CHUNK : (c + 1) * CHUNK],
                            start=True,
                            stop=True,
                        )
                    sl = slice(2 * p * CHUNK, (2 * p + 1) * CHUNK)
                    sl2 = slice((2 * p + 1) * CHUNK, (2 * p + 2) * CHUNK)
                    if p % 2 == 0:
                        nc.scalar.copy(u_sb[:, sl], ups[:, 0:CHUNK])
                        nc.vector.tensor_copy(u_sb[:, sl2], ups[:, CHUNK:])
                    else:
                        nc.vector.tensor_copy(u_sb[:, sl], ups[:, 0:CHUNK])
                        nc.scalar.copy(u_sb[:, sl2], ups[:, CHUNK:])

                def emit_v_group(g):
                    # v tiles 8g..8g+7 (x chunks 2g, 2g+1)
                    vps = ps_s.tile([C, 8 * D1], f32, tag="squad")
                    for u in range(8):
                        t = 8 * g + u
                        vsl = slice(u * D1, (u + 1) * D1)
                        nc.tensor.matmul(
                            vps[:, vsl], ones128, bvrow,
                            start=True, stop=False, skip_group_check=True,
                        )
                        nc.tensor.matmul(
                            vps[:, vsl], x_sb[:, t * 128 : (t + 1) * 128], wv_sb,
                            start=False, stop=True, skip_group_check=True,
                        )
                    nc.vector.tensor_copy(v_sb[:, 8 * g : 8 * g + 8, :], vps[:])
                    acols = v_sb[:, 8 * g : 8 * g + 8, D1 - 1]
                    nc.vector.tensor_scalar(
                        out=asch_sb[:, 8 * g : 8 * g + 8], in0=acols,
                        scalar1=SCH_A, scalar2=SCH_B, op0=Mult, op1=Add,
                    )
                    nc.gpsimd.tensor_scalar_mul(
                        aact_sb[:, 8 * g : 8 * g + 8], acols, SCALE
                    )

                def emit_sq(j, cp):
                    sq = ps_s.tile([128, 1024], f32, tag="squad")
                    xt = x_sb[:, j * 128 : (j + 1) * 128]
                    for i in range(2):
                        sl = slice((2 * cp + i) * CHUNK, (2 * cp + i + 1) * CHUNK)
                        nc.tensor.matmul(
                            sq[:, i * CHUNK : (i + 1) * CHUNK], xt, u_sb[:, sl],
                            start=True, stop=True,
                        )
                    return sq

                def emit_acc_clear(acc):
                    # one tiny full-partition matmul with start=True marks
                    # the whole acc bank pending-zero (writes only a spare
                    # col, never read); every real write then overwrites on
                    # first touch regardless of execution order
                    nc.tensor.matmul(
                        acc[:, 300:301], wp_sb, x_sb[:, 0:1],
                        start=True, stop=True, skip_group_check=True,
                    )

                def emit_exp_act(sq, j, et):
                    # j1 tile WHOLE in one ACT instruction (per-instruction
                    # sem+dispatch tax dominates fine splits; ACT reads sq1,
                    # the later tile, so its finish gates a slot 2 jps out)
                    nc.scalar.activation(
                        out=et[:, :], in_=sq[:, :], func=Exp,
                        scale=SCALE, bias=aact_sb[:, j : j + 1],
                    )

                def emit_exp_dve(sq, j, et):
                    # j0 tile WHOLE on DVE (GPSIMD cannot access PSUM, so
                    # exp is an ACT+DVE affair; one instr per tile)
                    nc.vector.tensor_scalar(
                        out=et[:, :].bitcast(i16), in0=sq[:, :],
                        scalar1=SCH_A, scalar2=asch_sb[:, j : j + 1],
                        op0=Mult, op1=Add,
                    )



                def emit_ev(acc, j0, j1, et0, et1):
                    # dependency tracking is per-TILE: each exp slice is its
                    # own tile so an ev block waits only its producer.
                    # All start=False (bank pre-marked by emit_acc_clear);
                    # adds commute via per-byte pending-zero semantics.
                    vt0 = v_sb[:, j0, :]
                    vt1 = v_sb[:, j1, :]
                    last = j1 == NKT - 1
                    for b in range(8):
                        nc.tensor.matmul(
                            acc[:, D1 * b : D1 * (b + 1)],
                            et1[:, 128 * b : 128 * (b + 1)], vt1,
                            start=False, stop=False, skip_group_check=True,
                        )
                    for b in range(8):
                        nc.tensor.matmul(
                            acc[:, D1 * b : D1 * (b + 1)],
                            et0[:, 128 * b : 128 * (b + 1)], vt0,
                            start=False, stop=(last and b == 7),
                            skip_group_check=True,
                        )

                def emit_recip8(acc_p):
                    # all 8 block sums -> reciprocals in one DVE op
                    r8 = episb.tile([128, 8], f32, tag="recip")
                    sums = acc_p[:, 0:144].rearrange("p (b c) -> p b c", c=D1)[
                        :, :, 0
                    ]
                    nc.vector.reciprocal(r8, sums)
                    return r8

                def emit_norm8(acc_p, r8, on8, q, nb=2):
                    # normalize nb blocks per DVE op
                    bs = slice(2 * q, 2 * q + nb)
                    blocks = acc_p[:, 0:144].rearrange(
                        "p (b c) -> p b c", c=D1
                    )[:, bs, 1:17]
                    rb = r8[:, bs].unsqueeze(2).broadcast_to((128, nb, 16))
                    nc.vector.scalar_tensor_tensor(
                        out=on8[:, bs, :], in0=blocks, scalar=1.0, in1=rb,
                        op0=Mult, op1=Mult,
                    )

                def emit_epi_bias(acc_p):
                    # pps[w_new, 16b+d] for all 8 blocks: bias via K=1 matmul
                    # (both operands f32r: BIR forbids mixing f32r with bf16)
                    nc.tensor.matmul(
                        acc_p[:, 144:272], cb[0:1, 0:128], cb[0:1, 128:256],
                        start=False, stop=False, skip_group_check=True,
                    )

                def emit_epi_proj(acc_p, on8, p):
                    # blocks 2p, 2p+1: out[w_new, d] += wp^T @ onorm
                    for i in range(2):
                        b = 2 * p + i
                        nc.tensor.matmul(
                            acc_p[:, 144 + 16 * b : 144 + 16 * (b + 1)],
                            wp_sb, on8[:, b, :],
                            start=False, stop=False, skip_group_check=True,
                        )

                def emit_epi_evac(acc_p, osb, half):
                    cs = slice(64 * half, 64 * half + 64)
                    nc.scalar.copy(
                        osb[:, cs], acc_p[:, 144 + 64 * half : 208 + 64 * half]
                    )

                def emit_epi_out(osb, cp_p):
                    nc.sync.dma_start(
                        out=out_d[:, cp_p * 128 : (cp_p + 1) * 128], in_=osb
                    )

                # ---- PE p-state warm-up: keep the tensor engine busy from
                # t~0.3us (memset source: no DMA dependency) so the clock is
                # at 2.4GHz when the main loop starts (ramp needs 3us of
                # continuous execution) ----

                # ---- prologue: ALL u/v up front (engines are idle; doing
                # this inside the loop collides with the saturated exp
                # pipeline and costs more than the serial prologue) ----
                for p in range(4):
                    emit_u_pair(p)
                    emit_v_group(p)
                pro = {}

                pend_ev = []  # queue of (acc, j0, j1, et0a, et0b, et1a, et1b)
                pend_epi = None  # (acc, cp)
                r8 = on8 = None
                for cp in range(NCP):
                    acc = ps_acc.tile([128, 512], f32, tag="acc")
                    emit_acc_clear(acc)
                    for jp in range(16):
                        j0, j1 = 2 * jp, 2 * jp + 1
                        # DVE carries the per-cp epilogue in small quanta on
                        # distinct jps; on those jps DVE's exp share shrinks
                        # (cols shift to Pool). Pool itself stays spike-free.

                        # epilogue pieces FIRST on DVE: in-order sequencers —
                        # anything emitted before the exp must already be
                        # ready or it head-of-line blocks the exp
                        if pend_epi is not None:
                            acc_p, cp_p = pend_epi
                            # ev runs 3 jps behind: acc(cp_p) is complete
                            # only after ev(15) emitted at jp2 -> epilogue
                            # reads start at jp3
                            if jp == 3:
                                r8 = emit_recip8(acc_p)
                                on8 = episb.tile([128, 8, 16], bf16, tag="onorm")
                                osb = episb.tile([128, 128], f32, tag="osb")
                            elif jp == 4:
                                emit_norm8(acc_p, r8, on8, 0, nb=4)
                            elif jp == 6:
                                emit_norm8(acc_p, r8, on8, 2, nb=4)
                            elif jp == 13:
                                emit_epi_evac(acc_p, osb, 0)
                            elif jp == 15:
                                emit_epi_evac(acc_p, osb, 1)
                                emit_epi_out(osb, cp_p)
                        sq0 = emit_sq(j0, cp)
                        et0 = etp.tile([128, 1024], bf16, tag="et0")
                        emit_exp_dve(sq0, j0, et0)
                        sq1 = emit_sq(j1, cp)
                        et1 = etp.tile([128, 1024], bf16, tag="et1")
                        emit_exp_act(sq1, j1, et1)
                        # ev runs TWO jps behind its exp: a full extra jp of
                        # slack absorbs all engine jitter / epilogue spikes
                        if len(pend_ev) == 3:
                            emit_ev(*pend_ev.pop(0))
                        if cp == 0 and jp in pro:
                            for f in pro[jp]:
                                f()
                        if pend_epi is not None and 7 <= jp <= 13 and jp % 2 == 1:
                            acc_p, cp_p = pend_epi
                            if jp == 7:
                                emit_epi_bias(acc_p)
                            emit_epi_proj(acc_p, on8, (jp - 7) // 2)
                        pend_ev.append((acc, j0, j1, et0, et1))
                    pend_epi = (acc, cp)

                # ---- tail: last evs + last epilogue ----
                for ev_args in pend_ev:
                    emit_ev(*ev_args)
                acc_p, cp_p = pend_epi
                r8 = emit_recip8(acc_p)
                on8 = episb.tile([128, 8, 16], bf16, tag="onorm")
                osb = episb.tile([128, 128], f32, tag="osb")
                emit_norm8(acc_p, r8, on8, 0, nb=4)
                emit_norm8(acc_p, r8, on8, 2, nb=4)
                emit_epi_bias(acc_p)
                for p in range(4):
                    emit_epi_proj(acc_p, on8, p)
                emit_epi_evac(acc_p, osb, 0)
                emit_epi_evac(acc_p, osb, 1)
                emit_epi_out(osb, cp_p)

    nc.compile()
    return nc


def _get_program():
    if "nc" not in _CACHE:
        _CACHE["nc"] = _build()
    return _CACHE["nc"]


def _make_in_maps(x, w_qkv, b_qkv, w_proj, b_proj):
    import ml_dtypes

    x_f = np.asarray(x, dtype=np.float32).reshape(C, L)
    x_cl = np.ascontiguousarray(x_f.astype(ml_dtypes.bfloat16))
    w_qkv = np.asarray(w_qkv, dtype=np.float32)
    b_qkv = np.asarray(b_qkv, dtype=np.float32)
    w_proj = np.asarray(w_proj, dtype=np.float32)
    b_proj = np.asarray(b_proj, dtype=np.float32)

    wpT = np.ascontiguousarray(w_proj.T)  # (w, w_new)

    in_maps = []
    for i in range(N_CORES):
        rows_q = np.arange(D) * 24 + i * 3 + 0  # d-major split of the 3C axis
        rows_k = rows_q + 1
        rows_v = rows_q + 2
        wq = w_qkv[rows_q]  # [16, 128]
        wk = w_qkv[rows_k]
        wv = w_qkv[rows_v]
        bq = b_qkv[rows_q]
        bk = b_qkv[rows_k]
        bv = b_qkv[rows_v]

        wkvb = np.zeros((C, WKW), dtype=np.float64)
        wkvb[:, _WP : _WP + 128] = wpT
        wkvb[:, _M : _M + 128] = wq.T.astype(np.float64) @ wk.astype(np.float64)
        wkvb[:, _WV + 1 : _WV + 17] = wv.T
        wkvb[:, _WV + 17] = wk.T @ bq  # A_raw weights
        wkvb[0, _ON128 : _ON128 + 128] = 1.0
        wkvb[0, _BVROW] = 1.0
        wkvb[0, _BVROW + 1 : _BVROW + 17] = bv
        wkvb[0, _BVROW + 17] = float(bq @ bk)
        wkvb[0, _ON16 : _ON16 + 16] = 1.0

        cb = np.zeros((1, 1024), dtype=np.float32)
        cb[0, 0:128] = b_proj  # bias column for the transposed projection
        cb[0, 128:256] = 1.0  # f32 ones row (bias matmul moving operand)

        in_maps.append(
            {
                "x_cl": x_cl,
                "wkvb": wkvb.astype(ml_dtypes.bfloat16),
                "cblob": cb,
            }
        )
    return in_maps


def _run(in_maps, trace=False):
    from concourse.bass_utils import run_bass_kernel_spmd

    nc = _get_program()
    return run_bass_kernel_spmd(nc, in_maps, list(range(N_CORES)), trace=trace)


def _assemble(results):
    out = np.empty((1, C, H, W), dtype=np.float32)
    for i in range(N_CORES):
        # out2[w, 16*h + d] -> y[d, h, w]
        o2 = results[i]["out2"].reshape(W, H, D)
        out[0, i * D : (i + 1) * D] = o2.transpose(2, 1, 0)
    return out


def kernel(x, w_qkv, b_qkv, w_proj, b_proj):
    in_maps = _make_in_maps(x, w_qkv, b_qkv, w_proj, b_proj)
    r = _run(in_maps, trace=False)
    return _assemble(r.results)


def kernel_with_timing(x, w_qkv, b_qkv, w_proj, b_proj):
    """Like kernel() but also returns an HW execution time estimate in ns.

    The axon client in this container has no NTFF profiling hook, so when
    hardware profiling is unavailable we fall back to the concourse
    cost-model timeline simulator (single core; cores are identical/independent).
    """
    in_maps = _make_in_maps(x, w_qkv, b_qkv, w_proj, b_proj)
    try:
        r = _run(in_maps, trace=True)
        exec_ns = r.exec_time_ns
    except ModuleNotFoundError:
        r = _run(in_maps, trace=False)
        exec_ns = None
    if exec_ns is None:
        exec_ns = _CACHE.get("tlsim_ns")
        if exec_ns is None:
            from concourse.timeline_sim import TimelineSim

            exec_ns = int(TimelineSim(_get_program()).simulate())
            _CACHE["tlsim_ns"] = exec_ns
    return _assemble(r.results), exec_ns
